# revision 2
# baseline (speedup 1.0000x reference)
"""LocalAttention TRN2 kernel: data-parallel over batch (8 batches/core x 8 cores).

Math per batch (see reference):
  t  = texts @ W_txt.T + b_txt        [l=512, dim=1024]
  im = images @ W_img.T + b_img       [n=1024, dim=1024]
  dots = (t @ im.T) * dim**-0.5       [l, n]
  norm = softmax(dots.flatten())      (max-subtraction skipped: |dots| < ~8,
                                       exp() is exact-safe in fp32 there)
  attn = norm.sum(axis=l)             [n]
  out  = attn[:, None] * im           [n, dim]

Device-side layout choices:
  - All matmul contractions need the contraction dim on the SBUF partition
    axis, so the host uploads imagesT [img_dim, n], textsT [txt_dim, l],
    W_txtT [txt_dim, dim], W_imgT [img_dim, dim]; projections then produce
    tT [dim, l] / imT [dim, n] directly and dots = tT.T-blocks @ imT.
  - `out` is produced transposed ([dim, n]) on device; host transposes back.
  - Matmul inputs in bf16 (full PE speed); accumulation + outputs fp32.
"""
import sys
sys.path.insert(0, '/opt/trn_rl_repo')

import numpy as np
import ml_dtypes

import concourse.bass as bass
import concourse.tile as tile
from concourse import bacc, mybir
from concourse.bass_utils import run_bass_kernel_spmd

BF = mybir.dt.bfloat16
F32 = mybir.dt.float32
AF = mybir.ActivationFunctionType

B, L, N_IMG, DIM, IMG_D, TXT_D = 64, 512, 1024, 1024, 1024, 768
SCALE = DIM ** (-0.5)
NCORES = 8
BPC = B // NCORES          # batches per core
KT_T = TXT_D // 128        # 6 k-tiles for text proj
KT_I = IMG_D // 128        # 8 k-tiles for image proj
DT = DIM // 128            # 8 dim tiles
LT = L // 128              # 4 l tiles
NCH = N_IMG // 512         # 2 n chunks of 512

_NC_CACHE = None


def _build_nc():
    nc = bacc.Bacc(None, target_bir_lowering=False)
    imgT_d = nc.dram_tensor("imagesT", [BPC, IMG_D, N_IMG], BF, kind="ExternalInput")
    txtT_d = nc.dram_tensor("textsT", [BPC, TXT_D, L], BF, kind="ExternalInput")
    wt_d = nc.dram_tensor("W_txtT", [TXT_D, DIM], BF, kind="ExternalInput")
    wi_d = nc.dram_tensor("W_imgT", [IMG_D, DIM], BF, kind="ExternalInput")
    btx_d = nc.dram_tensor("b_txt", [DIM], F32, kind="ExternalInput")
    bim_d = nc.dram_tensor("b_img", [DIM], F32, kind="ExternalInput")
    outT_d = nc.dram_tensor("outT", [BPC, DIM, N_IMG], F32, kind="ExternalOutput")
    nd_d = nc.dram_tensor("norm_dots", [BPC, L, N_IMG], F32, kind="ExternalOutput")
    la_d = nc.dram_tensor("local_attn", [BPC, N_IMG], F32, kind="ExternalOutput")

    with tile.TileContext(nc) as tc:
        with (
            tc.tile_pool(name="singles", bufs=1) as singles,
            tc.tile_pool(name="imgs", bufs=2) as imgs,
            tc.tile_pool(name="txts", bufs=2) as txts,
            tc.tile_pool(name="tts", bufs=2) as tts,
            tc.tile_pool(name="bigs", bufs=1) as bigs,
            tc.tile_pool(name="rows", bufs=4) as rows,
            tc.tile_pool(name="bcs", bufs=2) as bcs,
            tc.tile_pool(name="psmm", bufs=4, space="PSUM") as psmm,
            tc.tile_pool(name="pssm", bufs=2, space="PSUM") as pssm,
        ):
            # ---- weights / constants (resident) ----
            wt_sb = singles.tile([128, KT_T, DIM], BF)
            wi_sb = singles.tile([128, KT_I, DIM], BF)
            for kt in range(KT_T):
                nc.sync.dma_start(wt_sb[:, kt, :], wt_d[kt * 128:(kt + 1) * 128, :])
            for kt in range(KT_I):
                nc.sync.dma_start(wi_sb[:, kt, :], wi_d[kt * 128:(kt + 1) * 128, :])
            btx_sb = singles.tile([128, DT], F32)
            bim_sb = singles.tile([128, DT], F32)
            nc.sync.dma_start(btx_sb, btx_d[:].rearrange("(dt p) -> p dt", p=128))
            nc.sync.dma_start(bim_sb, bim_d[:].rearrange("(dt p) -> p dt", p=128))
            ones_col = singles.tile([128, 1], BF)     # colsum lhsT [K=128, M=1]
            nc.vector.memset(ones_col, 1.0)
            ones_row = singles.tile([1, 128], F32)    # broadcast lhsT [K=1, M=128]
            nc.vector.memset(ones_row, 1.0)

            for i in range(BPC):
                # ---- load activations (transposed on host) ----
                im_sb = imgs.tile([128, KT_I, N_IMG], BF, tag="im")
                for kt in range(KT_I):
                    nc.sync.dma_start(im_sb[:, kt, :],
                                      imgT_d[i, kt * 128:(kt + 1) * 128, :])
                tx_sb = txts.tile([128, KT_T, L], BF, tag="tx")
                for kt in range(KT_T):
                    nc.sync.dma_start(tx_sb[:, kt, :],
                                      txtT_d[i, kt * 128:(kt + 1) * 128, :])

                # ---- tT [dim, l] = W_txtT.T-blocks @ textsT + b_txt ----
                tT_sb = tts.tile([128, DT, L], BF, tag="tT")
                for dt in range(DT):
                    ps = psmm.tile([128, 512], F32, tag="mm")
                    for kt in range(KT_T):
                        nc.tensor.matmul(ps, wt_sb[:, kt, dt * 128:(dt + 1) * 128],
                                         tx_sb[:, kt, :],
                                         start=(kt == 0), stop=(kt == KT_T - 1))
                    nc.scalar.activation(tT_sb[:, dt, :], ps, AF.Identity,
                                         bias=btx_sb[:, dt:dt + 1])

                # ---- imT [dim, n] = W_imgT.T-blocks @ imagesT + b_img ----
                # fp32 copy (feeds `out`) + bf16 copy (feeds dots matmul)
                imT_sb = bigs.tile([128, DT, N_IMG], F32, tag="imT")
                imTb_sb = bigs.tile([128, DT, N_IMG], BF, tag="imTb")
                for dt in range(DT):
                    for nch in range(NCH):
                        ps = psmm.tile([128, 512], F32, tag="mm")
                        for kt in range(KT_I):
                            nc.tensor.matmul(
                                ps, wi_sb[:, kt, dt * 128:(dt + 1) * 128],
                                im_sb[:, kt, nch * 512:(nch + 1) * 512],
                                start=(kt == 0), stop=(kt == KT_I - 1))
                        sl = slice(nch * 512, (nch + 1) * 512)
                        nc.scalar.activation(imT_sb[:, dt, sl], ps, AF.Identity,
                                             bias=bim_sb[:, dt:dt + 1])
                        nc.vector.tensor_scalar_add(imTb_sb[:, dt, sl], ps,
                                                    bim_sb[:, dt:dt + 1])

                # ---- dots -> e = exp(SCALE * dots); eb = bf16(e) ----
                e_sb = bigs.tile([128, LT, N_IMG], F32, tag="e")
                eb_sb = bigs.tile([128, LT, N_IMG], BF, tag="eb")
                for lt in range(LT):
                    for nch in range(NCH):
                        ps = psmm.tile([128, 512], F32, tag="mm")
                        for kt in range(DT):
                            nc.tensor.matmul(
                                ps, tT_sb[:, kt, lt * 128:(lt + 1) * 128],
                                imTb_sb[:, kt, nch * 512:(nch + 1) * 512],
                                start=(kt == 0), stop=(kt == DT - 1))
                        sl = slice(nch * 512, (nch + 1) * 512)
                        nc.scalar.activation(e_sb[:, lt, sl], ps, AF.Exp,
                                             scale=SCALE)
                        nc.vector.tensor_copy(eb_sb[:, lt, sl], e_sb[:, lt, sl])

                # ---- local_attn raw = colsum_l(e)  (PE cross-partition sum) ----
                attn_raw = rows.tile([1, N_IMG], F32, tag="araw")
                for nch in range(NCH):
                    psa = pssm.tile([1, 512], F32, tag="sm")
                    for lt in range(LT):
                        nc.tensor.matmul(
                            psa, ones_col,
                            eb_sb[:, lt, nch * 512:(nch + 1) * 512],
                            start=(lt == 0), stop=(lt == LT - 1))
                    nc.vector.tensor_copy(attn_raw[:, nch * 512:(nch + 1) * 512],
                                          psa)

                # ---- S, inv, attn, broadcasts ----
                s_sb = rows.tile([1, 1], F32, tag="s")
                nc.vector.reduce_sum(s_sb, attn_raw, axis=mybir.AxisListType.X)
                inv_sb = rows.tile([1, 1], F32, tag="inv")
                nc.vector.reciprocal(inv_sb, s_sb)
                attn_sb = rows.tile([1, N_IMG], F32, tag="attn")
                nc.vector.tensor_scalar_mul(attn_sb, attn_raw, inv_sb)
                nc.sync.dma_start(la_d[i:i + 1, :], attn_sb)

                invb_ps = pssm.tile([128, 1], F32, tag="sm")
                nc.tensor.matmul(invb_ps, ones_row, inv_sb)
                inv_bc = rows.tile([128, 1], F32, tag="invbc")
                nc.vector.tensor_copy(inv_bc, invb_ps)

                attn_bc = bcs.tile([128, N_IMG], F32, tag="attnbc")
                for nch in range(NCH):
                    psb = pssm.tile([128, 512], F32, tag="sm")
                    nc.tensor.matmul(psb, ones_row,
                                     attn_sb[:, nch * 512:(nch + 1) * 512])
                    nc.vector.tensor_copy(attn_bc[:, nch * 512:(nch + 1) * 512],
                                          psb)

                # ---- norm_dots = e * inv (in place), DMA out ----
                for lt in range(LT):
                    nc.vector.tensor_scalar_mul(e_sb[:, lt, :], e_sb[:, lt, :],
                                                inv_bc)
                nc.sync.dma_start(
                    nd_d[i, :, :].rearrange("(lt p) n -> p lt n", p=128), e_sb)

                # ---- outT = imT * attn_bc (in place), DMA out ----
                for dt in range(DT):
                    nc.vector.tensor_mul(imT_sb[:, dt, :], imT_sb[:, dt, :],
                                         attn_bc)
                nc.sync.dma_start(
                    outT_d[i, :, :].rearrange("(dt p) n -> p dt n", p=128), imT_sb)

    nc.compile()
    nc.finalize()
    return nc


def kernel(images, texts, W_txt, b_txt, W_img, b_img):
    global _NC_CACHE
    if _NC_CACHE is None:
        _NC_CACHE = _build_nc()
    nc = _NC_CACHE

    bf = ml_dtypes.bfloat16
    imagesT = np.ascontiguousarray(images.transpose(0, 2, 1)).astype(bf)
    textsT = np.ascontiguousarray(texts.transpose(0, 2, 1)).astype(bf)
    W_txtT = np.ascontiguousarray(W_txt.T).astype(bf)
    W_imgT = np.ascontiguousarray(W_img.T).astype(bf)
    b_txt = np.ascontiguousarray(b_txt, dtype=np.float32)
    b_img = np.ascontiguousarray(b_img, dtype=np.float32)

    in_maps = []
    for c in range(NCORES):
        sl = slice(c * BPC, (c + 1) * BPC)
        in_maps.append({
            "imagesT": imagesT[sl], "textsT": textsT[sl],
            "W_txtT": W_txtT, "W_imgT": W_imgT,
            "b_txt": b_txt, "b_img": b_img,
        })

    res = run_bass_kernel_spmd(nc, in_maps, core_ids=list(range(NCORES)))

    out = np.empty((B, N_IMG, DIM), np.float32)
    norm_dots = np.empty((B, L, N_IMG), np.float32)
    local_attn = np.empty((B, N_IMG), np.float32)
    for c in range(NCORES):
        r = res.results[c]
        sl = slice(c * BPC, (c + 1) * BPC)
        out[sl] = r["outT"].transpose(0, 2, 1)
        norm_dots[sl] = r["norm_dots"]
        local_attn[sl] = r["local_attn"]
    return out, norm_dots, local_attn
